# revision 36
# baseline (speedup 1.0000x reference)
"""Trainium2 Bass kernel for a Mixtral decoder layer (attention + top-2 MoE).

Strategy (8 NeuronCores):
  Launch 1 (attention): 2D shard = (batch b in {0,1}) x (head-group g in {0..3},
    4 heads / 256 feature slice each). Each core computes q/k/v projections for
    its slice, transposed-scores flash-style attention (scores computed as
    s^T[tk, tq] so the softmax denominator folds into a ones-column of V), and
    a partial output projection. Host sums the 4 partials per batch.
  Host: residual add, rmsnorm, gating logits, exact top-2 routing, per-expert
    token gather (expert-parallel dispatch done in numpy - free).
  Launch 2 (MoE FFN): expert-parallel - core e owns expert e's w1/w3/w2 and
    processes its routed tokens (padded to capacity C) densely, pipelined over
    512-token blocks.
  Host: scatter-add expert outputs + residual. All matmuls bf16 with fp32 PSUM
    accumulation; softmax/normalization/routing in fp32.
"""
import os
import sys

import numpy as np
import ml_dtypes

for _p in ("/root/.axon_site", "/root/.axon_site/_ro/trn_rl_repo", "/opt/trn_rl_repo"):
    if os.path.isdir(_p) and _p not in sys.path:
        sys.path.append(_p)

import concourse.tile as tile
from concourse import bacc, mybir
from concourse.bass_utils import run_bass_kernel_spmd

BF16 = ml_dtypes.bfloat16
AF = mybir.ActivationFunctionType
ALU = mybir.AluOpType
DT = mybir.dt

H = 1024
S = 2048
B = 2
NH = 16
D = 64
E = 8
I = 2048
T = B * S
EPS = 1e-5

NCORES = 8
NGRP = 4              # head groups (cores per batch)
NHPC = NH // NGRP     # 4 heads per core
DS = NHPC * D         # 256-wide feature slice per core
TQC = 4               # tq chunks of 512
NTK = S // 128        # 16 tk tiles
NCI = H // 128        # 8 contraction chunks

C = 1088              # MoE expert token capacity (per-expert max on this data ~1087)
SW1, SW3, SW2 = 64.0, 16.0, 64.0   # fp8 weight scales (powers of 2, exact to undo)
F8 = ml_dtypes.float8_e4m3

_CACHE = {}
LAST_RESULTS = []     # BassKernelResults of the last kernel() call (for test harness)
TRACE = os.environ.get("KERNEL_TRACE", "0") == "1"

# tk tiles per attend-half whose softmax exp runs on DVE (custom fused op)
# instead of ACT, to balance the two engines
DVE_EXP_TKT = (5, 10, 15)
DVE_EXP_TKT_B = (2, 7, 12)


def _register_exp_ops():
    """Register two custom DVE ops computing exp via a degree-2 polynomial
    base and repeated squaring: op1 = (1 + w + w^2/2)^8 ~ e^{8w} with
    w = in*C0, op2 = x^4. Chained with C0 = scale/32 they give e^{scale*in}
    to ~1% relative accuracy on |scale*in| <= 4.2."""
    import concourse.dve_ops as dve_ops
    from concourse.dve_spec import Spec, Src0, C0, C1, One, sq, lower
    from concourse.dve_spec import _has_src1 as has_src1
    from concourse.dve_ops import DveOp, OPS, CUSTOM_DVE_SPECS, _SUB_OPCODE_FOR_NAME
    from concourse.dve_uop import DveOpSpec

    if "EXP_BASE_ANT" in CUSTOM_DVE_SPECS:
        return dve_ops.EXP_BASE_ANT, dve_ops.EXP_SQ2_ANT

    w = Src0 * C0
    p = One + w * (One + w * C1)

    def ref1(in0, in1, c0, c1, c2):
        ww = in0.astype(np.float32) * c0
        pp = 1.0 + ww * (1.0 + ww * c1)
        return (pp ** 8).astype(np.float32)

    def ref2(in0, in1, c0, c1, c2):
        x = in0.astype(np.float32)
        return (x * x) * (x * x)

    ops = []
    row = max(_SUB_OPCODE_FOR_NAME.values()) + 1
    for name, body, ref in (("EXP_BASE_ANT", sq(sq(sq(p))), ref1),
                            ("EXP_SQ2_ANT", sq(sq(Src0)), ref2)):
        spec = Spec(body=body, reference=ref)
        shas = {}
        for ver in ("v3", "v4"):
            s = DveOpSpec(name=name, opcode=row, uops=lower(spec, ver=ver),
                          rd1_en=has_src1(spec))
            shas[ver] = s.sha(ver)
        op = DveOp(name, spec, subdim=False, uops_sha=shas)
        _SUB_OPCODE_FOR_NAME[name] = row
        OPS.append(op)
        CUSTOM_DVE_SPECS[name] = spec
        setattr(dve_ops, name, op)
        ops.append(op)
        row += 1
    assert row <= 0x20
    return ops[0], ops[1]


def _capacity_chunks(cap):
    out, o = [], 0
    while o < cap:
        ln = min(512, cap - o)
        out.append((o, ln))
        o += ln
    return out


def _build_l1():
    """Attention, fp8-DoubleRow projections + flipped AV.

    Weights wq/wk/wv scaled by SQK=32 on host (fp8 range); q',k' = 32*true so
    scores = 1024*true, folded into the exp scale 2^-13. v' = 32*true; the
    AV output is 32*attn, normalized by the softmax denom (ones-column of v,
    unscaled), and 1/32 is folded into woT on host. AV is computed transposed:
    out[q_slice(128), d+1(65)] = pt[tk,q].T @ v[tk,65] so the denominator is a
    per-partition scalar and M=128 (full PE array)."""
    nc = bacc.Bacc("TRN2", target_bir_lowering=False, debug=False, num_devices=NCORES)
    xT8 = nc.dram_tensor("xT8", [H, S], DT.float8e4, kind="ExternalInput")
    wqkv8 = nc.dram_tensor("wqkv8", [H, 3 * DS], DT.float8e4, kind="ExternalInput")
    woT = nc.dram_tensor("woT", [DS, H], DT.bfloat16, kind="ExternalInput")
    h1p = nc.dram_tensor("h1p", [S, H], DT.bfloat16, kind="ExternalOutput")

    NPH = H // 256       # 4 H k-pairs for DoubleRow
    DR = mybir.MatmulPerfMode.DoubleRow
    EXPSC = 0.125 / (32.0 * 32.0)    # softmax 1/8 plus q,k weight scales
    EXP_BASE, EXP_SQ2 = _register_exp_ops()
    with tile.TileContext(nc) as tc:
        with tc.tile_pool(name="wpool", bufs=1) as wpool, \
             tc.tile_pool(name="qk", bufs=1) as qkpool, \
             tc.tile_pool(name="vp", bufs=1) as vpool, \
             tc.tile_pool(name="pt", bufs=4) as ptpool, \
             tc.tile_pool(name="ao", bufs=1) as aopool, \
             tc.tile_pool(name="at", bufs=3) as atpool, \
             tc.tile_pool(name="rc", bufs=4) as rcpool, \
             tc.tile_pool(name="st", bufs=3) as stpool, \
             tc.tile_pool(name="hout", bufs=4) as hpool, \
             tc.tile_pool(name="pp", bufs=2, space="PSUM") as pp, \
             tc.tile_pool(name="pav", bufs=2, space="PSUM") as pav:

            # ---- loads: 4 big DMAs (wqkv, x half 0, x half 1, wo) ----
            wqkv_t = wpool.tile([128, NPH, 2, 3 * DS], DT.float8e4)
            nc.sync.dma_start(
                wqkv_t[:], wqkv8.rearrange("(a two p) m -> p a two m", p=128, two=2))
            x8h = [wpool.tile([128, NPH, 2, S // 2], DT.float8e4, name=f"x8h{hf}",
                              tag=f"x8h{hf}") for hf in range(2)]
            xr = xT8.rearrange("(a two p) s -> p a two s", p=128, two=2)
            nc.sync.dma_start(x8h[0][:], xr[:, :, :, 0:S // 2])
            nc.sync.dma_start(x8h[1][:], xr[:, :, :, S // 2:S])
            wo_sb = wpool.tile([128, DS // 128, H], DT.bfloat16)
            nc.sync.dma_start(wo_sb[:], woT.rearrange("(c p) m -> p c m", p=128))

            # q/k per head-pair [128, S] bf16 (partitions 0:64 = even head's d,
            # 64:128 = odd head's; scaled by 32); v for all heads in one
            # [128, tk-pair, 2, head, 72] fp8 tile (col 64 = ones)
            qts = [[qkpool.tile([128, S // 2], DT.bfloat16, name=f"q{p}{th}",
                                tag=f"q{p}{th}") for th in range(2)]
                   for p in range(NHPC // 2)]
            kts = [[qkpool.tile([128, S // 2], DT.bfloat16, name=f"k{p}{th}",
                                tag=f"k{p}{th}") for th in range(2)]
                   for p in range(NHPC // 2)]
            vall = vpool.tile([128, NTK // 2, 2, NHPC, 72], DT.float8e4)
            nc.vector.memset(vall[:, :, :, :, 64:65], 1.0)
            aoTs = [aopool.tile([128, DS // 128, S // 2], DT.bfloat16,
                                name=f"aoT{hf}", tag=f"aoT{hf}") for hf in range(2)]

            def make_qk(pair, th, woff, dst, on_act=False):
                # dst[pair][th][128, 1024] bf16; partitions 0:64 even head,
                # 64:128 odd head of the pair; values 32x. woff: 0=q, DS=k.
                ps = pp.tile([128, 1024], DT.float32, tag="pp", name="ps")
                for i in range(2):
                    for p in range(NPH):
                        nc.tensor.matmul(
                            ps[:, i * 512:(i + 1) * 512],
                            wqkv_t[:, p, :, woff + pair * 128:woff + (pair + 1) * 128],
                            x8h[th][:, p, :, i * 512:(i + 1) * 512],
                            start=(p == 0), stop=(p == NPH - 1),
                            perf_mode=DR,
                        )
                if on_act:
                    nc.scalar.activation(dst[pair][th][:, :], ps[:, :], AF.Copy)
                else:
                    nc.vector.tensor_copy(dst[pair][th][:, :], ps[:, :])

            def make_v():
                for tkt in range(NTK):
                    pv = pp.tile([128, DS], DT.float32, tag="pp", name="pv")
                    for p in range(NPH):
                        nc.tensor.matmul(
                            pv[:, 0:DS],
                            x8h[tkt // 8][:, p, :, (tkt % 8) * 128:(tkt % 8 + 1) * 128],
                            wqkv_t[:, p, :, 2 * DS:3 * DS],
                            start=(p == 0), stop=(p == NPH - 1),
                            perf_mode=DR,
                        )
                    nc.vector.tensor_copy(
                        vall[:, tkt // 2, tkt % 2, :, 0:64],
                        pv[:, 0:DS].rearrange("p (h d) -> p h d", d=64))

            def av_mms(h, av, ptp, j):
                for qs in range(8):
                    nc.tensor.matmul(
                        av[:, qs, 0:65],
                        ptp[:, :, qs * 128:(qs + 1) * 128],
                        vall[:, j, :, h, 0:65],
                        start=(j == 0), stop=(j == NTK // 2 - 1),
                        perf_mode=DR,
                    )

            def attend2(h0, h1, half):
                # two heads interleaved at tk-tile granularity: each head's
                # scores have a full exp of pipeline slack, hidden behind the
                # other head's exp on ACT
                stt = {}
                for h in (h0, h1):
                    stt[h] = {
                        "av": pav.tile([128, 8, 128], DT.float32, tag="pav",
                                       name="av"),
                        "pending": None, "ptp": None,
                    }
                for tkt in range(NTK):
                    for h in (h0, h1):
                        d = stt[h]
                        qt, kt = qts[h // 2][half], kts[h // 2]
                        ro = (h % 2) * 64
                        sc = pp.tile([128, 1024], DT.float32, tag="pp", name="sc")
                        for i in range(2):
                            nc.tensor.matmul(
                                sc[:, i * 512:(i + 1) * 512],
                                kt[tkt // 8][ro:ro + 64,
                                             (tkt % 8) * 128:(tkt % 8 + 1) * 128],
                                qt[ro:ro + 64, i * 512:(i + 1) * 512],
                                start=True, stop=True,
                            )
                        if tkt % 2 == 0:
                            d["ptp"] = ptpool.tile([128, 2, 1024], DT.float8e4,
                                                   tag="pt", name="ptp")
                        dve_set = DVE_EXP_TKT if h % 2 == 0 else DVE_EXP_TKT_B
                        if tkt in dve_set:
                            st = stpool.tile([128, 1024], DT.bfloat16, tag="st")
                            nc.vector._custom_dve(EXP_BASE, out=st[:], in0=sc[:],
                                                  s0=EXPSC / 32.0, s1=0.5)
                            nc.vector._custom_dve(EXP_SQ2,
                                                  out=d["ptp"][:, tkt % 2, :],
                                                  in0=st[:])
                        else:
                            nc.scalar.activation(d["ptp"][:, tkt % 2, :], sc[:],
                                                 AF.Exp, scale=EXPSC)
                        if tkt % 2 == 1:
                            if d["pending"] is not None:
                                av_mms(h, d["av"], *d["pending"])
                            d["pending"] = (d["ptp"], tkt // 2)
                for h in (h0, h1):
                    d = stt[h]
                    av = d["av"]
                    av_mms(h, av, *d["pending"])
                    if h % 2 == 0:
                        at2[(h // 2, half)] = atpool.tile([128, 8, 128],
                                                          DT.bfloat16,
                                                          tag="at", name="at2")
                    at = at2[(h // 2, half)]
                    roff = (h % 2) * 64
                    rec = rcpool.tile([128, 8], DT.float32, tag="rc")
                    nc.vector.reciprocal(rec[:, :], av[:, :, 64])
                    nc.vector.tensor_tensor(
                        at[:, :, roff:roff + 64], av[:, :, 0:64],
                        rec[:, :, None].to_broadcast([128, 8, 64]), ALU.mult)

            def tp_pair(pair, half):
                # one blocked transpose: out[j, b, q] = at[q, b, j]
                at = at2.pop((pair, half))
                nc.sync.dma_start_transpose(
                    aoTs[half][:, pair, 0:1024].rearrange("p (b q) -> p b q", q=128),
                    at.rearrange("p a b -> p (a b)"))

            def oproj(half, ts, act_split=False):
                # h1p[tq, :] partial for the given tq tiles of this half
                for t in ts:
                    tkt = half * 8 + t
                    ht = hpool.tile([128, H], DT.bfloat16, tag="ht")
                    for jc in range(2):
                        po = pp.tile([128, 512], DT.float32, tag="pp", name="po")
                        for oc in range(2):
                            nc.tensor.matmul(
                                po[:, :],
                                aoTs[half][:, oc, t * 128:(t + 1) * 128],
                                wo_sb[:, oc, jc * 512:(jc + 1) * 512],
                                start=(oc == 0), stop=(oc == 1),
                            )
                        if act_split and jc == 1:
                            nc.scalar.activation(ht[:, jc * 512:(jc + 1) * 512],
                                                 po[:], AF.Copy)
                        else:
                            nc.vector.tensor_copy(ht[:, jc * 512:(jc + 1) * 512],
                                                  po[:])
                    nc.sync.dma_start(h1p[tkt * 128:(tkt + 1) * 128, :], ht[:])

            at2 = {}
            make_qk(0, 0, 0, qts)
            make_qk(0, 0, DS, kts, on_act=True)   # ACT: parallel startup
            make_qk(0, 1, 0, qts)
            make_qk(0, 1, DS, kts, on_act=True)
            make_qk(1, 0, 0, qts)
            make_qk(1, 0, DS, kts)
            make_qk(1, 1, 0, qts)
            make_qk(1, 1, DS, kts)
            make_v()
            attend2(0, 1, 0)
            tp_pair(0, 0)
            attend2(2, 3, 0)
            tp_pair(1, 0)
            # oproj(0) interleaved with half-1 attends so PE never displaces
            # the ACT-critical scores for long
            attend2(0, 1, 1)
            oproj(0, range(0, 4))
            tp_pair(0, 1)
            attend2(2, 3, 1)
            oproj(0, range(4, 8))
            tp_pair(1, 1)
            oproj(1, range(8), act_split=True)

    nc.compile()
    nc.finalize()
    return nc


def _build_l2(cap):
    """Expert-parallel MoE FFN, fp8e4m3 + DoubleRow (2 K-tiles/instruction).

    Scales: w1 *= SW1 (silu scale=1/SW1 undoes exactly), w3 *= SW3,
    w2 *= SW2; web divided by SW3*SW2 on host. hh = silu_true * SW3*g_true
    stays well under fp8e4 max 240 for SW3=16."""
    nc = bacc.Bacc("TRN2", target_bir_lowering=False, debug=False, num_devices=NCORES)
    zeT = nc.dram_tensor("zeT", [H, cap], DT.float8e4, kind="ExternalInput")
    w1T = nc.dram_tensor("w1T", [H, I], DT.float8e4, kind="ExternalInput")
    w3T = nc.dram_tensor("w3T", [H, I], DT.float8e4, kind="ExternalInput")
    w2T = nc.dram_tensor("w2T", [I, H], DT.float8e4, kind="ExternalInput")
    web = nc.dram_tensor("web", [128, cap], DT.float32, kind="ExternalInput")
    yT = nc.dram_tensor("yT", [H, cap], DT.bfloat16, kind="ExternalOutput")

    cch = _capacity_chunks(cap)
    NIC = I // 128
    NPH = H // 256        # 4 K-pairs over H
    NPI = I // 256        # 8 K-pairs over I
    DR = mybir.MatmulPerfMode.DoubleRow
    with tile.TileContext(nc) as tc:
        with tc.tile_pool(name="wpool", bufs=1) as wpool, \
             tc.tile_pool(name="hh", bufs=1) as hhpool, \
             tc.tile_pool(name="hs", bufs=3) as hspool, \
             tc.tile_pool(name="yt", bufs=3) as ytpool, \
             tc.tile_pool(name="pg", bufs=4, space="PSUM") as pg, \
             tc.tile_pool(name="py", bufs=4, space="PSUM") as py:

            # K-pair tiles; w1/w3 split in I-halves, loads interleaved per
            # pair so the h-phase streams behind the DMAs
            zps = [wpool.tile([128, 2, cap], DT.float8e4, name=f"zp{p}",
                              tag=f"zp{p}") for p in range(NPH)]
            w1ps = [[wpool.tile([128, 2, I // 2], DT.float8e4, name=f"w1p{p}_{b}",
                                tag=f"w1p{p}_{b}") for b in range(2)]
                    for p in range(NPH)]
            w3ps = [[wpool.tile([128, 2, I // 2], DT.float8e4, name=f"w3p{p}_{b}",
                                tag=f"w3p{p}_{b}") for b in range(2)]
                    for p in range(NPH)]
            zr = zeT.rearrange("(a two p) m -> a p two m", p=128, two=2)
            w1r = w1T.rearrange("(a two p) m -> a p two m", p=128, two=2)
            w3r = w3T.rearrange("(a two p) m -> a p two m", p=128, two=2)
            for p in range(NPH):
                nc.sync.dma_start(zps[p][:], zr[p])
                nc.sync.dma_start(w1ps[p][0][:], w1r[p][:, :, 0:I // 2])
                nc.sync.dma_start(w3ps[p][0][:], w3r[p][:, :, 0:I // 2])
            for p in range(NPH):
                nc.sync.dma_start(w1ps[p][1][:], w1r[p][:, :, I // 2:I])
                nc.sync.dma_start(w3ps[p][1][:], w3r[p][:, :, I // 2:I])
            web_sb = wpool.tile([128, cap], DT.float32)
            nc.sync.dma_start(web_sb[:], web[:, :])

            # hh as K-pair tiles over I for the DoubleRow y-phase
            hhp = [hhpool.tile([128, 2, cap], DT.float8e4, name=f"hhp{p}",
                               tag=f"hhp{p}") for p in range(NPI)]
            w2_holder = []

            for ic in range(NIC):
                b, bo = ic // 8, (ic % 8) * 128
                hs = hspool.tile([128, cap], DT.float8e4, tag="hs", name="hs")
                for j, (o, ln) in enumerate(cch):
                    hp = pg.tile([128, 512], DT.float32, tag="pg", name="hp")
                    for p in range(NPH):
                        nc.tensor.matmul(
                            hp[:, 0:ln],
                            w1ps[p][b][:, :, bo:bo + 128],
                            zps[p][:, :, o:o + ln],
                            start=(p == 0), stop=(p == NPH - 1),
                            perf_mode=DR,
                        )
                    nc.scalar.activation(hs[:, o:o + ln], hp[:, 0:ln], AF.Silu,
                                         scale=1.0 / SW1)
                    gp = pg.tile([128, 512], DT.float32, tag="pg", name="gp")
                    for p in range(NPH):
                        nc.tensor.matmul(
                            gp[:, 0:ln],
                            w3ps[p][b][:, :, bo:bo + 128],
                            zps[p][:, :, o:o + ln],
                            start=(p == 0), stop=(p == NPH - 1),
                            perf_mode=DR,
                        )
                    nc.vector.tensor_tensor(
                        hhp[ic // 2][:, ic % 2, o:o + ln],
                        gp[:, 0:ln], hs[:, o:o + ln], ALU.mult)
                if ic == 0:
                    # emit w2 load after the first h-block for DMA priority
                    w2ps = [wpool.tile([128, 2, H], DT.float8e4, name=f"w2p{p}",
                                       tag=f"w2p{p}") for p in range(NPI)]
                    w2r = w2T.rearrange("(a two p) m -> a p two m", p=128, two=2)
                    for p in range(NPI):
                        nc.sync.dma_start(w2ps[p][:], w2r[p])
                    w2_holder.append(w2ps)

            w2ps = w2_holder[0]
            for hc in range(NCI):
                yt = ytpool.tile([128, cap], DT.bfloat16, tag="yt", name="yt")
                for j, (o, ln) in enumerate(cch):
                    yp = py.tile([128, 512], DT.float32, tag="py", name="yp")
                    for p in range(NPI):
                        nc.tensor.matmul(
                            yp[:, 0:ln],
                            w2ps[p][:, :, hc * 128:(hc + 1) * 128],
                            hhp[p][:, :, o:o + ln],
                            start=(p == 0), stop=(p == NPI - 1),
                            perf_mode=DR,
                        )
                    nc.vector.tensor_tensor(
                        yt[:, o:o + ln], yp[:, 0:ln], web_sb[:, o:o + ln], ALU.mult)
                nc.sync.dma_start(yT[hc * 128:(hc + 1) * 128, :], yt[:])

    nc.compile()
    nc.finalize()
    return nc


def _get(name, builder, *args):
    if name not in _CACHE:
        _CACHE[name] = builder(*args)
    return _CACHE[name]


def _rmsnorm(x, w):
    xf = x.astype(np.float32)
    rms = 1.0 / np.sqrt((xf * xf).mean(axis=-1, keepdims=True) + EPS)
    return (xf * rms) * w.astype(np.float32)


def kernel(x, ln1_w, ln2_w, wq, wk, wv, wo, gate_w, w1, w2, w3):
    global LAST_RESULTS
    LAST_RESULTS = []
    x = np.asarray(x, np.float32)
    wq, wk, wv, wo = (np.asarray(a, np.float32) for a in (wq, wk, wv, wo))
    gate_w = np.asarray(gate_w, np.float32)
    w1, w2, w3 = (np.asarray(a, np.float32) for a in (w1, w2, w3))
    ln1_w = np.asarray(ln1_w, np.float32)
    ln2_w = np.asarray(ln2_w, np.float32)

    xf = x.reshape(T, H)
    z1 = _rmsnorm(xf, ln1_w)
    # ---- launch 1: attention (fp8 projections, bf16 scores/O-proj) ----
    SQK = 32.0
    nc1 = _get("l1", _build_l1)
    z1_8 = np.clip(z1, -240, 240).astype(F8)
    in_maps = []
    for c in range(NCORES):
        b, g = divmod(c, NGRP)
        sl = slice(g * DS, (g + 1) * DS)
        wqkv = np.concatenate([wq[sl].T, wk[sl].T, wv[sl].T], axis=1) * SQK
        in_maps.append({
            "xT8": np.ascontiguousarray(z1_8[b * S:(b + 1) * S].T),
            "wqkv8": np.clip(np.ascontiguousarray(wqkv), -240, 240).astype(F8),
            "woT": (np.ascontiguousarray(wo[:, sl].T) / SQK).astype(BF16),
        })
    res1 = run_bass_kernel_spmd(nc1, in_maps, core_ids=list(range(NCORES)), trace=TRACE)
    LAST_RESULTS.append(res1)

    h1 = xf.copy()
    for c in range(NCORES):
        b = c // NGRP
        h1[b * S:(b + 1) * S] += res1.results[c]["h1p"].astype(np.float32)

    # ---- host: routing (exact fp32 semantics like the reference) ----
    z = _rmsnorm(h1, ln2_w)
    logits = (z.astype(np.float64) @ gate_w.T.astype(np.float64)).astype(np.float32)
    order = np.argsort(-logits, axis=-1, kind="stable")
    sel = order[:, :2]                               # top-2, ties -> lower index
    vals = np.take_along_axis(logits, sel, axis=-1).astype(np.float32)
    mx = vals.max(axis=-1, keepdims=True)
    ex = np.exp(vals - mx)
    rw = (ex / ex.sum(axis=-1, keepdims=True)).astype(np.float32)

    idx_lists = []
    for e in range(E):
        m = (sel == e)
        tok = np.nonzero(m.any(axis=-1))[0]
        wgt = np.where(m, rw, 0.0).sum(axis=-1)[tok]
        idx_lists.append((tok, wgt.astype(np.float32)))
    maxload = max(len(tok) for tok, _ in idx_lists)
    cap = C
    while cap < maxload:
        cap += 512
    nc2 = _get(f"l2_{cap}", _build_l2, cap)

    # ---- launch 2: expert-parallel FFN (fp8) ----
    zT = np.clip(np.ascontiguousarray(z.T), -240, 240).astype(F8)    # [H, T]
    in_maps2 = []
    for e in range(E):
        tok, wgt = idx_lists[e]
        zeT = np.zeros((H, cap), F8)
        zeT[:, :len(tok)] = zT[:, tok]
        web = np.zeros((cap,), np.float32)
        web[:len(tok)] = wgt / (SW3 * SW2)
        in_maps2.append({
            "zeT": zeT,
            "w1T": np.clip(np.ascontiguousarray(w1[e].T) * SW1, -240, 240).astype(F8),
            "w3T": np.clip(np.ascontiguousarray(w3[e].T) * SW3, -240, 240).astype(F8),
            "w2T": np.clip(np.ascontiguousarray(w2[e].T) * SW2, -240, 240).astype(F8),
            "web": np.broadcast_to(web, (128, cap)).copy(),
        })
    res2 = run_bass_kernel_spmd(nc2, in_maps2, core_ids=list(range(NCORES)), trace=TRACE)
    LAST_RESULTS.append(res2)

    out = h1.copy()
    for e in range(E):
        tok, _ = idx_lists[e]
        out[tok] += res2.results[e]["yT"][:, :len(tok)].T.astype(np.float32)

    return out.reshape(B, S, H).astype(np.float32)



# revision 37
# speedup vs baseline: 1.0585x; 1.0585x over previous
"""Trainium2 Bass kernel for a Mixtral decoder layer (attention + top-2 MoE).

Strategy (8 NeuronCores):
  Launch 1 (attention): 2D shard = (batch b in {0,1}) x (head-group g in {0..3},
    4 heads / 256 feature slice each). Each core computes q/k/v projections for
    its slice, transposed-scores flash-style attention (scores computed as
    s^T[tk, tq] so the softmax denominator folds into a ones-column of V), and
    a partial output projection. Host sums the 4 partials per batch.
  Host: residual add, rmsnorm, gating logits, exact top-2 routing, per-expert
    token gather (expert-parallel dispatch done in numpy - free).
  Launch 2 (MoE FFN): expert-parallel - core e owns expert e's w1/w3/w2 and
    processes its routed tokens (padded to capacity C) densely, pipelined over
    512-token blocks.
  Host: scatter-add expert outputs + residual. All matmuls bf16 with fp32 PSUM
    accumulation; softmax/normalization/routing in fp32.
"""
import os
import sys

import numpy as np
import ml_dtypes

for _p in ("/root/.axon_site", "/root/.axon_site/_ro/trn_rl_repo", "/opt/trn_rl_repo"):
    if os.path.isdir(_p) and _p not in sys.path:
        sys.path.append(_p)

import concourse.tile as tile
from concourse import bacc, mybir
from concourse.bass_utils import run_bass_kernel_spmd

BF16 = ml_dtypes.bfloat16
AF = mybir.ActivationFunctionType
ALU = mybir.AluOpType
DT = mybir.dt

H = 1024
S = 2048
B = 2
NH = 16
D = 64
E = 8
I = 2048
T = B * S
EPS = 1e-5

NCORES = 8
NGRP = 4              # head groups (cores per batch)
NHPC = NH // NGRP     # 4 heads per core
DS = NHPC * D         # 256-wide feature slice per core
TQC = 4               # tq chunks of 512
NTK = S // 128        # 16 tk tiles
NCI = H // 128        # 8 contraction chunks

C = 1088              # MoE expert token capacity (per-expert max on this data ~1087)
SW1, SW3, SW2 = 64.0, 16.0, 64.0   # fp8 weight scales (powers of 2, exact to undo)
F8 = ml_dtypes.float8_e4m3

_CACHE = {}
LAST_RESULTS = []     # BassKernelResults of the last kernel() call (for test harness)
TRACE = os.environ.get("KERNEL_TRACE", "0") == "1"

# tk tiles per attend-half whose softmax exp runs on DVE (custom fused op)
# instead of ACT, to balance the two engines
DVE_EXP_TKT = (5, 10, 15)
DVE_EXP_TKT_B = (2, 7, 12)


def _register_exp_ops():
    """Register two custom DVE ops computing exp via a degree-2 polynomial
    base and repeated squaring: op1 = (1 + w + w^2/2)^8 ~ e^{8w} with
    w = in*C0, op2 = x^4. Chained with C0 = scale/32 they give e^{scale*in}
    to ~1% relative accuracy on |scale*in| <= 4.2."""
    import concourse.dve_ops as dve_ops
    from concourse.dve_spec import Spec, Src0, C0, C1, One, sq, lower
    from concourse.dve_spec import _has_src1 as has_src1
    from concourse.dve_ops import DveOp, OPS, CUSTOM_DVE_SPECS, _SUB_OPCODE_FOR_NAME
    from concourse.dve_uop import DveOpSpec

    if "EXP_BASE_ANT" in CUSTOM_DVE_SPECS:
        return dve_ops.EXP_BASE_ANT, dve_ops.EXP_SQ2_ANT

    w = Src0 * C0
    p = One + w * (One + w * C1)

    def ref1(in0, in1, c0, c1, c2):
        ww = in0.astype(np.float32) * c0
        pp = 1.0 + ww * (1.0 + ww * c1)
        return (pp ** 8).astype(np.float32)

    def ref2(in0, in1, c0, c1, c2):
        x = in0.astype(np.float32)
        return (x * x) * (x * x)

    ops = []
    row = max(_SUB_OPCODE_FOR_NAME.values()) + 1
    for name, body, ref in (("EXP_BASE_ANT", sq(sq(sq(p))), ref1),
                            ("EXP_SQ2_ANT", sq(sq(Src0)), ref2)):
        spec = Spec(body=body, reference=ref)
        shas = {}
        for ver in ("v3", "v4"):
            s = DveOpSpec(name=name, opcode=row, uops=lower(spec, ver=ver),
                          rd1_en=has_src1(spec))
            shas[ver] = s.sha(ver)
        op = DveOp(name, spec, subdim=False, uops_sha=shas)
        _SUB_OPCODE_FOR_NAME[name] = row
        OPS.append(op)
        CUSTOM_DVE_SPECS[name] = spec
        setattr(dve_ops, name, op)
        ops.append(op)
        row += 1
    assert row <= 0x20
    return ops[0], ops[1]


def _capacity_chunks(cap):
    out, o = [], 0
    while o < cap:
        ln = min(512, cap - o)
        out.append((o, ln))
        o += ln
    return out


def _build_l1():
    """Attention, fp8-DoubleRow projections + flipped AV.

    Weights wq/wk/wv scaled by SQK=32 on host (fp8 range); q',k' = 32*true so
    scores = 1024*true, folded into the exp scale 2^-13. v' = 32*true; the
    AV output is 32*attn, normalized by the softmax denom (ones-column of v,
    unscaled), and 1/32 is folded into woT on host. AV is computed transposed:
    out[q_slice(128), d+1(65)] = pt[tk,q].T @ v[tk,65] so the denominator is a
    per-partition scalar and M=128 (full PE array)."""
    nc = bacc.Bacc("TRN2", target_bir_lowering=False, debug=False, num_devices=NCORES)
    xT8 = nc.dram_tensor("xT8", [H, S], DT.float8e4, kind="ExternalInput")
    wqkv8 = nc.dram_tensor("wqkv8", [H, 3 * DS], DT.float8e4, kind="ExternalInput")
    woT = nc.dram_tensor("woT", [DS, H], DT.bfloat16, kind="ExternalInput")
    h1p = nc.dram_tensor("h1p", [S, H], DT.bfloat16, kind="ExternalOutput")

    NPH = H // 256       # 4 H k-pairs for DoubleRow
    DR = mybir.MatmulPerfMode.DoubleRow
    EXPSC = 0.125 / (32.0 * 32.0)    # softmax 1/8 plus q,k weight scales
    EXP_BASE, EXP_SQ2 = _register_exp_ops()
    with tile.TileContext(nc) as tc:
        with tc.tile_pool(name="wpool", bufs=1) as wpool, \
             tc.tile_pool(name="qk", bufs=1) as qkpool, \
             tc.tile_pool(name="vp", bufs=1) as vpool, \
             tc.tile_pool(name="pt", bufs=4) as ptpool, \
             tc.tile_pool(name="ao", bufs=1) as aopool, \
             tc.tile_pool(name="at", bufs=3) as atpool, \
             tc.tile_pool(name="rc", bufs=4) as rcpool, \
             tc.tile_pool(name="st", bufs=3) as stpool, \
             tc.tile_pool(name="hout", bufs=4) as hpool, \
             tc.tile_pool(name="pp", bufs=2, space="PSUM") as pp, \
             tc.tile_pool(name="pav", bufs=1, space="PSUM") as pav, \
             tc.tile_pool(name="ppo", bufs=2, space="PSUM") as ppo:

            # ---- loads: 4 big DMAs (wqkv, x half 0, x half 1, wo) ----
            wqkv_t = wpool.tile([128, NPH, 2, 3 * DS], DT.float8e4)
            nc.sync.dma_start(
                wqkv_t[:], wqkv8.rearrange("(a two p) m -> p a two m", p=128, two=2))
            x8h = [wpool.tile([128, NPH, 2, S // 2], DT.float8e4, name=f"x8h{hf}",
                              tag=f"x8h{hf}") for hf in range(2)]
            xr = xT8.rearrange("(a two p) s -> p a two s", p=128, two=2)
            nc.sync.dma_start(x8h[0][:], xr[:, :, :, 0:S // 2])
            nc.sync.dma_start(x8h[1][:], xr[:, :, :, S // 2:S])
            wo_sb = wpool.tile([128, DS // 128, H], DT.bfloat16)
            nc.sync.dma_start(wo_sb[:], woT.rearrange("(c p) m -> p c m", p=128))

            # q/k per head-pair [128, S] bf16 (partitions 0:64 = even head's d,
            # 64:128 = odd head's; scaled by 32); v for all heads in one
            # [128, tk-pair, 2, head, 72] fp8 tile (col 64 = ones)
            qts = [[qkpool.tile([128, S // 2], DT.bfloat16, name=f"q{p}{th}",
                                tag=f"q{p}{th}") for th in range(2)]
                   for p in range(NHPC // 2)]
            kts = [[qkpool.tile([128, S // 2], DT.bfloat16, name=f"k{p}{th}",
                                tag=f"k{p}{th}") for th in range(2)]
                   for p in range(NHPC // 2)]
            vall = vpool.tile([128, NTK // 2, 2, NHPC, 72], DT.float8e4)
            nc.vector.memset(vall[:, :, :, :, 64:65], 1.0)
            aoTs = [aopool.tile([128, DS // 128, S // 2], DT.bfloat16,
                                name=f"aoT{hf}", tag=f"aoT{hf}") for hf in range(2)]

            def make_qk(pair, th, woff, dst, on_act=False):
                # dst[pair][th][128, 1024] bf16; partitions 0:64 even head,
                # 64:128 odd head of the pair; values 32x. woff: 0=q, DS=k.
                ps = pp.tile([128, 1024], DT.float32, tag="pp", name="ps")
                for i in range(2):
                    for p in range(NPH):
                        nc.tensor.matmul(
                            ps[:, i * 512:(i + 1) * 512],
                            wqkv_t[:, p, :, woff + pair * 128:woff + (pair + 1) * 128],
                            x8h[th][:, p, :, i * 512:(i + 1) * 512],
                            start=(p == 0), stop=(p == NPH - 1),
                            perf_mode=DR,
                        )
                if on_act:
                    nc.scalar.activation(dst[pair][th][:, :], ps[:, :], AF.Copy)
                else:
                    nc.vector.tensor_copy(dst[pair][th][:, :], ps[:, :])

            def make_v():
                for tkt in range(NTK):
                    pv = ppo.tile([128, DS], DT.float32, tag="ppo", name="pv")
                    for p in range(NPH):
                        nc.tensor.matmul(
                            pv[:, 0:DS],
                            x8h[tkt // 8][:, p, :, (tkt % 8) * 128:(tkt % 8 + 1) * 128],
                            wqkv_t[:, p, :, 2 * DS:3 * DS],
                            start=(p == 0), stop=(p == NPH - 1),
                            perf_mode=DR,
                        )
                    nc.vector.tensor_copy(
                        vall[:, tkt // 2, tkt % 2, :, 0:64],
                        pv[:, 0:DS].rearrange("p (h d) -> p h d", d=64))

            def av_mms(h, av, ptp, j):
                for qs in range(8):
                    nc.tensor.matmul(
                        av[:, qs, 0:65],
                        ptp[:, :, qs * 128:(qs + 1) * 128],
                        vall[:, j, :, h, 0:65],
                        start=(j == 0), stop=(j == NTK // 2 - 1),
                        perf_mode=DR,
                    )

            def attend(h, half):
                # one tq half (1024 queries, 8 slices of 128) of head h;
                # av[q_slice, qs, 0:64] = unnormalized attn (32x), [.., 64] = denom
                qt, kt = qts[h // 2][half], kts[h // 2]
                ro = (h % 2) * 64
                av = pav.tile([128, 8, 128], DT.float32, tag="pav", name="av")
                pending = None
                ptp = None
                for tkt in range(NTK):
                    sc = pp.tile([128, 1024], DT.float32, tag="pp", name="sc")
                    for i in range(2):
                        nc.tensor.matmul(
                            sc[:, i * 512:(i + 1) * 512],
                            kt[tkt // 8][ro:ro + 64, (tkt % 8) * 128:(tkt % 8 + 1) * 128],
                            qt[ro:ro + 64, i * 512:(i + 1) * 512],
                            start=True, stop=True,
                        )
                    if tkt % 2 == 0:
                        ptp = ptpool.tile([128, 2, 1024], DT.float8e4, tag="pt")
                    if tkt in DVE_EXP_TKT:
                        st = stpool.tile([128, 1024], DT.bfloat16, tag="st")
                        nc.vector._custom_dve(EXP_BASE, out=st[:], in0=sc[:],
                                              s0=EXPSC / 32.0, s1=0.5)
                        nc.vector._custom_dve(EXP_SQ2, out=ptp[:, tkt % 2, :],
                                              in0=st[:])
                    else:
                        nc.scalar.activation(ptp[:, tkt % 2, :], sc[:], AF.Exp,
                                             scale=EXPSC)
                    if tkt % 2 == 1:
                        if pending is not None:
                            av_mms(h, av, *pending)
                        pending = (ptp, tkt // 2)
                av_mms(h, av, *pending)
                # two heads of a pair share an at2 tile: cols (h%2)*64..+64;
                # transposed into aoTs after the odd head (see tp_pair)
                if h % 2 == 0:
                    at2[(h // 2, half)] = atpool.tile([128, 8, 128], DT.bfloat16,
                                                      tag="at", name="at2")
                at = at2[(h // 2, half)]
                roff = (h % 2) * 64
                rec = rcpool.tile([128, 8], DT.float32, tag="rc")
                nc.vector.reciprocal(rec[:, :], av[:, :, 64])
                nc.vector.tensor_tensor(
                    at[:, :, roff:roff + 64], av[:, :, 0:64],
                    rec[:, :, None].to_broadcast([128, 8, 64]), ALU.mult)

            def tp_pair(pair, half):
                # one blocked transpose: out[j, b, q] = at[q, b, j]
                at = at2.pop((pair, half))
                nc.sync.dma_start_transpose(
                    aoTs[half][:, pair, 0:1024].rearrange("p (b q) -> p b q", q=128),
                    at.rearrange("p a b -> p (a b)"))

            def oproj(half, ts, act_split=False):
                # h1p[tq, :] partial for the given tq tiles of this half
                for t in ts:
                    tkt = half * 8 + t
                    ht = hpool.tile([128, H], DT.bfloat16, tag="ht")
                    for jc in range(2):
                        po = ppo.tile([128, 512], DT.float32, tag="ppo", name="po")
                        for oc in range(2):
                            nc.tensor.matmul(
                                po[:, :],
                                aoTs[half][:, oc, t * 128:(t + 1) * 128],
                                wo_sb[:, oc, jc * 512:(jc + 1) * 512],
                                start=(oc == 0), stop=(oc == 1),
                            )
                        if act_split and jc == 1:
                            nc.scalar.activation(ht[:, jc * 512:(jc + 1) * 512],
                                                 po[:], AF.Copy)
                        else:
                            nc.vector.tensor_copy(ht[:, jc * 512:(jc + 1) * 512],
                                                  po[:])
                    nc.sync.dma_start(h1p[tkt * 128:(tkt + 1) * 128, :], ht[:])

            at2 = {}
            make_qk(0, 0, 0, qts)
            make_qk(0, 0, DS, kts, on_act=True)   # ACT: parallel startup
            make_qk(0, 1, 0, qts)
            make_qk(0, 1, DS, kts, on_act=True)
            make_qk(1, 0, 0, qts)
            make_qk(1, 0, DS, kts)
            make_qk(1, 1, 0, qts)
            make_qk(1, 1, DS, kts)
            make_v()
            attend(0, 0)
            attend(1, 0)
            tp_pair(0, 0)
            attend(2, 0)
            attend(3, 0)
            tp_pair(1, 0)
            # oproj(0) interleaved with half-1 attends so PE never displaces
            # the ACT-critical scores for long
            attend(0, 1)
            oproj(0, range(0, 3))
            attend(1, 1)
            oproj(0, range(3, 6))
            tp_pair(0, 1)
            attend(2, 1)
            oproj(0, range(6, 8))
            attend(3, 1)
            tp_pair(1, 1)
            oproj(1, range(8), act_split=True)

    nc.compile()
    nc.finalize()
    return nc


def _build_l2(cap):
    """Expert-parallel MoE FFN, fp8e4m3 + DoubleRow (2 K-tiles/instruction).

    Scales: w1 *= SW1 (silu scale=1/SW1 undoes exactly), w3 *= SW3,
    w2 *= SW2; web divided by SW3*SW2 on host. hh = silu_true * SW3*g_true
    stays well under fp8e4 max 240 for SW3=16."""
    nc = bacc.Bacc("TRN2", target_bir_lowering=False, debug=False, num_devices=NCORES)
    zeT = nc.dram_tensor("zeT", [H, cap], DT.float8e4, kind="ExternalInput")
    w1T = nc.dram_tensor("w1T", [H, I], DT.float8e4, kind="ExternalInput")
    w3T = nc.dram_tensor("w3T", [H, I], DT.float8e4, kind="ExternalInput")
    w2T = nc.dram_tensor("w2T", [I, H], DT.float8e4, kind="ExternalInput")
    web = nc.dram_tensor("web", [128, cap], DT.float32, kind="ExternalInput")
    yT = nc.dram_tensor("yT", [H, cap], DT.bfloat16, kind="ExternalOutput")

    cch = _capacity_chunks(cap)
    NIC = I // 128
    NPH = H // 256        # 4 K-pairs over H
    NPI = I // 256        # 8 K-pairs over I
    DR = mybir.MatmulPerfMode.DoubleRow
    with tile.TileContext(nc) as tc:
        with tc.tile_pool(name="wpool", bufs=1) as wpool, \
             tc.tile_pool(name="hh", bufs=1) as hhpool, \
             tc.tile_pool(name="hs", bufs=3) as hspool, \
             tc.tile_pool(name="yt", bufs=3) as ytpool, \
             tc.tile_pool(name="pg", bufs=4, space="PSUM") as pg, \
             tc.tile_pool(name="py", bufs=4, space="PSUM") as py:

            # K-pair tiles; w1/w3 split in I-halves, loads interleaved per
            # pair so the h-phase streams behind the DMAs
            zps = [wpool.tile([128, 2, cap], DT.float8e4, name=f"zp{p}",
                              tag=f"zp{p}") for p in range(NPH)]
            w1ps = [[wpool.tile([128, 2, I // 2], DT.float8e4, name=f"w1p{p}_{b}",
                                tag=f"w1p{p}_{b}") for b in range(2)]
                    for p in range(NPH)]
            w3ps = [[wpool.tile([128, 2, I // 2], DT.float8e4, name=f"w3p{p}_{b}",
                                tag=f"w3p{p}_{b}") for b in range(2)]
                    for p in range(NPH)]
            zr = zeT.rearrange("(a two p) m -> a p two m", p=128, two=2)
            w1r = w1T.rearrange("(a two p) m -> a p two m", p=128, two=2)
            w3r = w3T.rearrange("(a two p) m -> a p two m", p=128, two=2)
            for p in range(NPH):
                nc.sync.dma_start(zps[p][:], zr[p])
                nc.sync.dma_start(w1ps[p][0][:], w1r[p][:, :, 0:I // 2])
                nc.sync.dma_start(w3ps[p][0][:], w3r[p][:, :, 0:I // 2])
            for p in range(NPH):
                nc.sync.dma_start(w1ps[p][1][:], w1r[p][:, :, I // 2:I])
                nc.sync.dma_start(w3ps[p][1][:], w3r[p][:, :, I // 2:I])
            web_sb = wpool.tile([128, cap], DT.float32)
            nc.sync.dma_start(web_sb[:], web[:, :])

            # hh as K-pair tiles over I for the DoubleRow y-phase
            hhp = [hhpool.tile([128, 2, cap], DT.float8e4, name=f"hhp{p}",
                               tag=f"hhp{p}") for p in range(NPI)]
            w2_holder = []

            for ic in range(NIC):
                b, bo = ic // 8, (ic % 8) * 128
                hs = hspool.tile([128, cap], DT.float8e4, tag="hs", name="hs")
                for j, (o, ln) in enumerate(cch):
                    hp = pg.tile([128, 512], DT.float32, tag="pg", name="hp")
                    for p in range(NPH):
                        nc.tensor.matmul(
                            hp[:, 0:ln],
                            w1ps[p][b][:, :, bo:bo + 128],
                            zps[p][:, :, o:o + ln],
                            start=(p == 0), stop=(p == NPH - 1),
                            perf_mode=DR,
                        )
                    nc.scalar.activation(hs[:, o:o + ln], hp[:, 0:ln], AF.Silu,
                                         scale=1.0 / SW1)
                    gp = pg.tile([128, 512], DT.float32, tag="pg", name="gp")
                    for p in range(NPH):
                        nc.tensor.matmul(
                            gp[:, 0:ln],
                            w3ps[p][b][:, :, bo:bo + 128],
                            zps[p][:, :, o:o + ln],
                            start=(p == 0), stop=(p == NPH - 1),
                            perf_mode=DR,
                        )
                    nc.vector.tensor_tensor(
                        hhp[ic // 2][:, ic % 2, o:o + ln],
                        gp[:, 0:ln], hs[:, o:o + ln], ALU.mult)
                if ic == 0:
                    # emit w2 load after the first h-block for DMA priority
                    w2ps = [wpool.tile([128, 2, H], DT.float8e4, name=f"w2p{p}",
                                       tag=f"w2p{p}") for p in range(NPI)]
                    w2r = w2T.rearrange("(a two p) m -> a p two m", p=128, two=2)
                    for p in range(NPI):
                        nc.sync.dma_start(w2ps[p][:], w2r[p])
                    w2_holder.append(w2ps)

            w2ps = w2_holder[0]
            for hc in range(NCI):
                yt = ytpool.tile([128, cap], DT.bfloat16, tag="yt", name="yt")
                for j, (o, ln) in enumerate(cch):
                    yp = py.tile([128, 512], DT.float32, tag="py", name="yp")
                    for p in range(NPI):
                        nc.tensor.matmul(
                            yp[:, 0:ln],
                            w2ps[p][:, :, hc * 128:(hc + 1) * 128],
                            hhp[p][:, :, o:o + ln],
                            start=(p == 0), stop=(p == NPI - 1),
                            perf_mode=DR,
                        )
                    nc.vector.tensor_tensor(
                        yt[:, o:o + ln], yp[:, 0:ln], web_sb[:, o:o + ln], ALU.mult)
                nc.sync.dma_start(yT[hc * 128:(hc + 1) * 128, :], yt[:])

    nc.compile()
    nc.finalize()
    return nc


def _get(name, builder, *args):
    if name not in _CACHE:
        _CACHE[name] = builder(*args)
    return _CACHE[name]


def _rmsnorm(x, w):
    xf = x.astype(np.float32)
    rms = 1.0 / np.sqrt((xf * xf).mean(axis=-1, keepdims=True) + EPS)
    return (xf * rms) * w.astype(np.float32)


def kernel(x, ln1_w, ln2_w, wq, wk, wv, wo, gate_w, w1, w2, w3):
    global LAST_RESULTS
    LAST_RESULTS = []
    x = np.asarray(x, np.float32)
    wq, wk, wv, wo = (np.asarray(a, np.float32) for a in (wq, wk, wv, wo))
    gate_w = np.asarray(gate_w, np.float32)
    w1, w2, w3 = (np.asarray(a, np.float32) for a in (w1, w2, w3))
    ln1_w = np.asarray(ln1_w, np.float32)
    ln2_w = np.asarray(ln2_w, np.float32)

    xf = x.reshape(T, H)
    z1 = _rmsnorm(xf, ln1_w)
    # ---- launch 1: attention (fp8 projections, bf16 scores/O-proj) ----
    SQK = 32.0
    nc1 = _get("l1", _build_l1)
    z1_8 = np.clip(z1, -240, 240).astype(F8)
    in_maps = []
    for c in range(NCORES):
        b, g = divmod(c, NGRP)
        sl = slice(g * DS, (g + 1) * DS)
        wqkv = np.concatenate([wq[sl].T, wk[sl].T, wv[sl].T], axis=1) * SQK
        in_maps.append({
            "xT8": np.ascontiguousarray(z1_8[b * S:(b + 1) * S].T),
            "wqkv8": np.clip(np.ascontiguousarray(wqkv), -240, 240).astype(F8),
            "woT": (np.ascontiguousarray(wo[:, sl].T) / SQK).astype(BF16),
        })
    res1 = run_bass_kernel_spmd(nc1, in_maps, core_ids=list(range(NCORES)), trace=TRACE)
    LAST_RESULTS.append(res1)

    h1 = xf.copy()
    for c in range(NCORES):
        b = c // NGRP
        h1[b * S:(b + 1) * S] += res1.results[c]["h1p"].astype(np.float32)

    # ---- host: routing (exact fp32 semantics like the reference) ----
    z = _rmsnorm(h1, ln2_w)
    logits = (z.astype(np.float64) @ gate_w.T.astype(np.float64)).astype(np.float32)
    order = np.argsort(-logits, axis=-1, kind="stable")
    sel = order[:, :2]                               # top-2, ties -> lower index
    vals = np.take_along_axis(logits, sel, axis=-1).astype(np.float32)
    mx = vals.max(axis=-1, keepdims=True)
    ex = np.exp(vals - mx)
    rw = (ex / ex.sum(axis=-1, keepdims=True)).astype(np.float32)

    idx_lists = []
    for e in range(E):
        m = (sel == e)
        tok = np.nonzero(m.any(axis=-1))[0]
        wgt = np.where(m, rw, 0.0).sum(axis=-1)[tok]
        idx_lists.append((tok, wgt.astype(np.float32)))
    maxload = max(len(tok) for tok, _ in idx_lists)
    cap = C
    while cap < maxload:
        cap += 512
    nc2 = _get(f"l2_{cap}", _build_l2, cap)

    # ---- launch 2: expert-parallel FFN (fp8) ----
    zT = np.clip(np.ascontiguousarray(z.T), -240, 240).astype(F8)    # [H, T]
    in_maps2 = []
    for e in range(E):
        tok, wgt = idx_lists[e]
        zeT = np.zeros((H, cap), F8)
        zeT[:, :len(tok)] = zT[:, tok]
        web = np.zeros((cap,), np.float32)
        web[:len(tok)] = wgt / (SW3 * SW2)
        in_maps2.append({
            "zeT": zeT,
            "w1T": np.clip(np.ascontiguousarray(w1[e].T) * SW1, -240, 240).astype(F8),
            "w3T": np.clip(np.ascontiguousarray(w3[e].T) * SW3, -240, 240).astype(F8),
            "w2T": np.clip(np.ascontiguousarray(w2[e].T) * SW2, -240, 240).astype(F8),
            "web": np.broadcast_to(web, (128, cap)).copy(),
        })
    res2 = run_bass_kernel_spmd(nc2, in_maps2, core_ids=list(range(NCORES)), trace=TRACE)
    LAST_RESULTS.append(res2)

    out = h1.copy()
    for e in range(E):
        tok, _ = idx_lists[e]
        out[tok] += res2.results[e]["yT"][:, :len(tok)].T.astype(np.float32)

    return out.reshape(B, S, H).astype(np.float32)

